# revision 13
# baseline (speedup 1.0000x reference)
"""GQA attention block (RoPE + causal attention + output proj) on 8 TRN2 NeuronCores.

Sharding: batch (B=2) x kv-head-group (KVH=4) -> 8 cores.
Core c handles batch b=c//4, kv group g=c%4 (q heads 4g..4g+3, kv head g).
Per-core tensor-parallel attention; AllGather of per-head outputs within each
batch's 4-core group; column-split wo after the gather.

All matmuls run in bf16 (fp32 PSUM accumulation). Layouts are transposed
([feature, token]) so Q/K/V projections, scores (computed as S^T = K-stationary),
and PV all feed the TensorEngine without transposes; softmax runs without
max-subtraction (logits are provably small for this problem's scale).

Scheduling notes:
- All SBUF pools coexist (phase-1 scratch must not alias phase-3 tiles, which
  would serialize attention behind the whole RoPE chain via WAR deps).
- Inputs are host-reshaped to [128, chunk, *] so each weight is one DMA
  descriptor and xT is eight column slices (the Sync queue's ~0.6us/descriptor
  issue rate was pacing phase 1).
- RoPE's rotate-half runs on the PE via a signed permutation matrix (prot);
  the DVE then does one mul per cos/sin term instead of four partition-sliced
  muls, halving the serial DVE chain that gates early attention.
- PV consumes both heads of a pair in a single matmul (shared V lhsT, 2D free
  AP), halving PV instruction/LDWEIGHTS overhead.
- Gathered-output DMAs issue from the (otherwise idle) GpSimd queue.
- The last span exchanges each (head-pair, half) quarter in its own AllGather
  as soon as its normalization lands, and the output projection accumulates
  per gathered quarter, so the exposed tail is one 64KB AllGather.
"""

import sys

if "/opt/trn_rl_repo" not in sys.path:
    sys.path.insert(0, "/opt/trn_rl_repo")

import numpy as np
import ml_dtypes

import concourse.bass as bass
import concourse.mybir as mybir
import concourse.tile as tile
from concourse import bacc
from concourse.bass_utils import run_bass_kernel_spmd

BF16 = ml_dtypes.bfloat16

B, S, HID = 2, 2048, 1024
H, KVH, D = 16, 4, 64
G = H // KVH
N_CORES = 8
SPAN = 512
NSPAN = S // SPAN  # 4
NCH = HID // 128  # 8 contraction chunks
NKT = S // 128  # 16 k-tiles
F32 = mybir.dt.float32
BF = mybir.dt.bfloat16

TRACE = False
_CACHED = {}


def _build_nc():
    nc = bacc.Bacc("TRN2", target_bir_lowering=False, debug=False, num_devices=N_CORES)

    xT = nc.dram_tensor("xT", [128, NCH, S], BF, kind="ExternalInput")
    wq = nc.dram_tensor("wq", [128, NCH, 256], BF, kind="ExternalInput")
    wkv = nc.dram_tensor("wkv", [128, NCH, 128], BF, kind="ExternalInput")
    wo = nc.dram_tensor("wo", [128, NCH, 256], BF, kind="ExternalInput")
    c2 = nc.dram_tensor("c2", [128, S], BF, kind="ExternalInput")
    s2 = nc.dram_tensor("s2", [128, S], F32, kind="ExternalInput")
    c1 = nc.dram_tensor("c1", [64, S], BF, kind="ExternalInput")
    s1 = nc.dram_tensor("s1", [64, S], F32, kind="ExternalInput")
    ident = nc.dram_tensor("ident", [128, 128], BF, kind="ExternalInput")
    dmask = nc.dram_tensor("dmask", [128, 128], BF, kind="ExternalInput")
    prot = nc.dram_tensor("prot", [128, 128], BF, kind="ExternalInput")
    out = nc.dram_tensor("out", [256, S], BF, kind="ExternalOutput")

    EXP = mybir.ActivationFunctionType.Exp

    with tile.TileContext(nc) as tc:
        with (
            tc.tile_pool(name="main", bufs=1) as main,
            tc.tile_pool(name="ropep", bufs=2) as ropep,
            tc.tile_pool(name="pp", bufs=9) as pp,
            tc.tile_pool(name="work", bufs=2) as work,
            tc.tile_pool(name="dramp", bufs=1, space="DRAM") as dramp,
        ):
            # Warmup collective: absorbs the collective-stream bootstrap
            # barrier while phase-1 compute runs.
            warm_in = dramp.tile([1, 64], BF, name="warm_in")
            warm_out = dramp.tile([4, 64], BF, name="warm_out")
            nc.gpsimd.collective_compute(
                "AllGather",
                mybir.AluOpType.bypass,
                replica_groups=[[0, 1, 2, 3], [4, 5, 6, 7]],
                ins=[warm_in[:].opt()],
                outs=[warm_out[:].opt()],
            )
            # ---- persistent SBUF ----
            xT_sb = main.tile([128, NCH, S], BF, name="xT_sb")
            wq_sb = main.tile([128, NCH, 256], BF, name="wq_sb")
            wkv_sb = main.tile([128, NCH, 128], BF, name="wkv_sb")
            wo_sb = main.tile([128, NCH, 256], BF, name="wo_sb")
            c2_sb = main.tile([128, S], BF, name="c2_sb")
            s2_sb = main.tile([128, S], F32, name="s2_sb")
            c1_sb = main.tile([64, S], BF, name="c1_sb")
            s1_sb = main.tile([64, S], F32, name="s1_sb")
            ident_sb = main.tile([128, 128], BF, name="ident_sb")
            dmask_sb = main.tile([128, 128], BF, name="dmask_sb")
            prot_sb = main.tile([128, 128], BF, name="prot_sb")
            qT0_sb = main.tile([128, S], BF, name="qT0_sb")
            qT1_sb = main.tile([128, S], BF, name="qT1_sb")
            kT2_sb = main.tile([128, S], BF, name="kT2_sb")
            vT_sb = main.tile([64, S], BF, name="vT_sb")
            vaug_sb = main.tile([128, NKT, 65], BF, name="vaug_sb")
            ones_sb = main.tile([1, 64], BF, name="ones_sb")
            qT_sb = [qT0_sb, qT1_sb]

            HS = S // 2  # phase-1 half-sequence granularity (2 PSUM banks)
            # DMA order = earliest-consumer-first; xT streams in column slices
            # so the KV projection starts as soon as its operand cols land.
            for cs in range(2):
                nc.sync.dma_start(
                    xT_sb[:, :, 256 * cs : 256 * (cs + 1)],
                    xT[:, :, 256 * cs : 256 * (cs + 1)],
                )
            nc.sync.dma_start(wkv_sb[:], wkv[:])
            nc.sync.dma_start(c1_sb[:], c1[:])
            nc.sync.dma_start(s1_sb[:], s1[:])
            nc.sync.dma_start(prot_sb[:], prot[:])
            for cs in range(2, 4):
                nc.sync.dma_start(
                    xT_sb[:, :, 256 * cs : 256 * (cs + 1)],
                    xT[:, :, 256 * cs : 256 * (cs + 1)],
                )
            nc.sync.dma_start(wq_sb[:], wq[:])
            nc.sync.dma_start(c2_sb[:], c2[:])
            nc.sync.dma_start(s2_sb[:], s2[:])
            nc.sync.dma_start(ident_sb[:], ident[:])
            for cs in range(4, 8):
                nc.sync.dma_start(
                    xT_sb[:, :, 256 * cs : 256 * (cs + 1)],
                    xT[:, :, 256 * cs : 256 * (cs + 1)],
                )
            nc.sync.dma_start(dmask_sb[:], dmask[:])
            nc.sync.dma_start(wo_sb[:], wo[:])
            nc.vector.memset(ones_sb[:], 1.0)

            # ---- phase 1: projections (transposed layout) + RoPE; KV first so
            # the V-transpose can run while the Q projections are still going.
            # rotate-half is a PE matmul against prot; DVE does 3 ops per
            # 512-col slice (cos mul, sin mul from the rotated PSUM, add). ----
            with (
                tc.tile_pool(name="psA", bufs=2, space="PSUM") as psA,
                tc.tile_pool(name="psT", bufs=2, space="PSUM") as psT,
                tc.tile_pool(name="psR", bufs=2, space="PSUM") as psR,
            ):

                def rope(dst, src_sb, nparts, f0, cosT, sinT, tag):
                    # dst[:, f0:f0+HS] = src*cos + (prot^T @ src)*sin
                    for sp in range(2):
                        c0, c1_ = f0 + SPAN * sp, f0 + SPAN * (sp + 1)
                        rot = psR.tile(
                            [nparts, SPAN], F32, tag="rot", name=f"rot{tag}{sp}"
                        )
                        nc.tensor.matmul(
                            rot[:],
                            prot_sb[0:nparts, 0:nparts],
                            src_sb[:, SPAN * sp : SPAN * (sp + 1)],
                            start=True,
                            stop=True,
                        )
                        tcos = ropep.tile(
                            [nparts, SPAN], BF, tag="tcos", name=f"tc{tag}{sp}"
                        )
                        tsin = ropep.tile(
                            [nparts, SPAN], BF, tag="tsin", name=f"ts{tag}{sp}"
                        )
                        nc.vector.tensor_mul(
                            tcos[:],
                            src_sb[:, SPAN * sp : SPAN * (sp + 1)],
                            cosT[0:nparts, c0:c1_],
                        )
                        nc.vector.tensor_mul(tsin[:], rot[:], sinT[0:nparts, c0:c1_])
                        nc.vector.tensor_add(dst[0:nparts, c0:c1_], tcos[:], tsin[:])

                for hf in range(2):
                    f0 = HS * hf
                    kvp = psA.tile([128, HS], F32, tag="qkv", name=f"kvp{hf}")
                    for sp in range(2):
                        for k in range(NCH):
                            nc.tensor.matmul(
                                kvp[:, SPAN * sp : SPAN * (sp + 1)],
                                wkv_sb[:, k, :],
                                xT_sb[:, k, f0 + SPAN * sp : f0 + SPAN * (sp + 1)],
                                start=(k == 0),
                                stop=(k == NCH - 1),
                            )
                    kb = ropep.tile([64, HS], BF, tag="kb", name=f"kb{hf}")
                    nc.scalar.copy(kb[:], kvp[0:64, :])
                    nc.scalar.copy(vT_sb[:, f0 : f0 + HS], kvp[64:128, :])
                    rope(kT2_sb, kb, 64, f0, c1_sb, s1_sb, f"k{hf}")
                    nc.vector.tensor_copy(
                        kT2_sb[64:128, f0 : f0 + HS], kT2_sb[0:64, f0 : f0 + HS]
                    )
                    # V transpose to [token, d] for this half
                    for t in range(8 * hf, 8 * hf + 8):
                        trp = psT.tile([128, 64], BF, tag="tr", name=f"tr{t}")
                        nc.tensor.transpose(
                            trp[:],
                            vT_sb[:, 128 * t : 128 * (t + 1)],
                            ident_sb[0:64, 0:64],
                        )
                        nc.vector.tensor_copy(vaug_sb[:, t, 0:64], trp[:])
                    for p in range(2):
                        qp = psA.tile([128, HS], F32, tag="qkv", name=f"qp{p}_{hf}")
                        for sp in range(2):
                            for k in range(NCH):
                                nc.tensor.matmul(
                                    qp[:, SPAN * sp : SPAN * (sp + 1)],
                                    wq_sb[:, k, 128 * p : 128 * (p + 1)],
                                    xT_sb[:, k, f0 + SPAN * sp : f0 + SPAN * (sp + 1)],
                                    start=(k == 0),
                                    stop=(k == NCH - 1),
                                )
                        qb = ropep.tile([128, HS], BF, tag="qb", name=f"qb{p}{hf}")
                        nc.scalar.copy(qb[:], qp[:])
                        rope(qT_sb[p], qb, 128, f0, c2_sb, s2_sb, f"q{p}{hf}")
                nc.vector.memset(vaug_sb[:, :, 64:65], 1.0)

            # ---- phase 3: attention spans, AllGather, output projection ----
            with (
                tc.tile_pool(name="psS", bufs=2, space="PSUM") as psS,
                tc.tile_pool(name="psO", bufs=1, space="PSUM") as psO,
            ):
                rg = [[0, 1, 2, 3], [4, 5, 6, 7]]
                pending_oproj = []

                # last-span per-quarter exchange buffers; piece q=2*pr+hh holds
                # its 4 gathered 64-row blocks at partition base 64*hh so
                # oproj's lhsT/rhs bases match
                agin3 = [
                    dramp.tile([64, SPAN], BF, name=f"agin3_{q}") for q in range(4)
                ]
                agout3 = [
                    dramp.tile([256, SPAN], BF, name=f"agout3_{q}") for q in range(4)
                ]
                of3 = [
                    work.tile([128, 4, SPAN], BF, tag="of3", bufs=4, name=f"of3_{q}")
                    for q in range(4)
                ]

                for J in range(NSPAN):
                    q0 = SPAN * J
                    nkt_j = 4 * (J + 1)
                    split_ag = J == NSPAN - 1
                    if not split_ag:
                        agin = dramp.tile([256, SPAN], BF, name=f"agin{J}")
                        agout = dramp.tile([4 * 256, SPAN], BF, name=f"agout{J}")

                    pending_norm = None
                    for pr in range(2):
                        opsum = psO.tile(
                            [128, 2 * SPAN], F32, tag=f"o{pr}", name=f"opsum{J}_{pr}"
                        )
                        src = qT_sb[pr]
                        pv_queue = []

                        def emit_pv(j, pt, off, pr=pr, opsum=opsum, nkt_j=nkt_j):
                            # matmul output is capped at one PSUM bank (512
                            # fp32/partition), so PV stays per-head
                            for hh in range(2):
                                nc.tensor.matmul(
                                    opsum[0:65, SPAN * hh + off : SPAN * (hh + 1)],
                                    vaug_sb[:, j, :],
                                    pt[:, SPAN * hh + off : SPAN * (hh + 1)],
                                    start=(j == 0),
                                    stop=(j == nkt_j - 1),
                                )

                        for jb in range(0, nkt_j, 2):
                            batch = []
                            for j in range(jb, min(jb + 2, nkt_j)):
                                jj = j - 4 * J
                                off = 128 * jj if jj > 0 else 0
                                sps = psS.tile(
                                    [128, 2 * SPAN], F32, tag="s", name=f"s{J}_{j}_{pr}"
                                )
                                pt = pp.tile(
                                    [128, 2 * SPAN], BF, tag="p", name=f"p{J}_{j}_{pr}"
                                )
                                for hh in range(2):
                                    nc.tensor.matmul(
                                        sps[
                                            :, SPAN * hh + off : SPAN * (hh + 1)
                                        ],
                                        kT2_sb[
                                            64 * hh : 64 * (hh + 1),
                                            128 * j : 128 * (j + 1),
                                        ],
                                        src[
                                            64 * hh : 64 * (hh + 1),
                                            q0 + off : q0 + SPAN,
                                        ],
                                        start=True,
                                        stop=True,
                                    )
                                batch.append((j, sps, pt, off))
                            for j, sps, pt, off in batch:
                                if off == 0:
                                    nc.scalar.activation(pt[:, :], sps[:, :], EXP)
                                else:
                                    for hh in range(2):
                                        nc.scalar.activation(
                                            pt[:, SPAN * hh + off : SPAN * (hh + 1)],
                                            sps[:, SPAN * hh + off : SPAN * (hh + 1)],
                                            EXP,
                                        )
                                jj = j - 4 * J
                                if jj >= 0:
                                    for hh in range(2):
                                        nc.vector.tensor_mul(
                                            pt[
                                                :,
                                                SPAN * hh + off : SPAN * hh
                                                + off
                                                + 128,
                                            ],
                                            pt[
                                                :,
                                                SPAN * hh + off : SPAN * hh
                                                + off
                                                + 128,
                                            ],
                                            dmask_sb[:],
                                        )
                                pv_queue.append((j, pt, off))
                            while len(pv_queue) > 4:
                                emit_pv(*pv_queue.pop(0))
                                emit_pv(*pv_queue.pop(0))
                        for args in pv_queue:
                            emit_pv(*args)

                        # normalization: denominator row copy (DVE, the one
                        # allowed PSUM operand), PE broadcast, reciprocal into
                        # SBUF (DVE), then the normalizing multiply. For pr0 of
                        # non-split spans the chain is deferred until after
                        # pr1's attention so the PE never waits on the handoff.
                        dsb = work.tile(
                            [1, 2 * SPAN], BF, tag="dsb", name=f"dsb{J}_{pr}"
                        )
                        nc.vector.tensor_copy(dsb[:], opsum[64:65, :])

                        def do_norm(
                            J=J,
                            pr=pr,
                            opsum=opsum,
                            dsb=dsb,
                            split_ag=split_ag,
                        ):
                          for hh in range(2):
                            bc = psS.tile(
                                [64, SPAN], F32, tag="s", name=f"bc{J}_{pr}_{hh}"
                            )
                            nc.tensor.matmul(
                                bc[:],
                                ones_sb[:],
                                dsb[0:1, SPAN * hh : SPAN * (hh + 1)],
                                start=True,
                                stop=True,
                            )
                            rec = work.tile(
                                [64, SPAN], F32, tag="rec", name=f"rec{J}_{pr}_{hh}"
                            )
                            nc.vector.reciprocal_approx_fast(rec[:], bc[:])
                            onrm = work.tile(
                                [64, SPAN],
                                BF,
                                tag="onrm",
                                bufs=8,
                                name=f"on{J}_{pr}_{hh}",
                            )
                            nc.vector.tensor_mul(
                                onrm[:],
                                opsum[0:64, SPAN * hh : SPAN * (hh + 1)],
                                rec[:],
                            )
                            if split_ag:
                                q = 2 * pr + hh
                                nc.sync.dma_start(agin3[q][:], onrm[:])
                                nc.gpsimd.collective_compute(
                                    "AllGather",
                                    mybir.AluOpType.bypass,
                                    replica_groups=rg,
                                    ins=[agin3[q][:].opt()],
                                    outs=[agout3[q][:].opt()],
                                )
                                for g in range(4):
                                    nc.sync.dma_start(
                                        of3[q][64 * hh : 64 * hh + 64, g, :],
                                        agout3[q][64 * g : 64 * (g + 1), :],
                                    )
                            else:
                                nc.sync.dma_start(
                                    agin[
                                        128 * pr + 64 * hh : 128 * pr
                                        + 64 * (hh + 1),
                                        :,
                                    ],
                                    onrm[:],
                                )

                        if split_ag or pr == 1:
                            if pending_norm is not None:
                                pending_norm()
                                pending_norm = None
                            do_norm()
                        else:
                            pending_norm = do_norm

                        # cover the last span's AG latency with the two still-
                        # pending full-span output projections
                        if split_ag and pending_oproj:
                            pending_oproj.pop(0)()

                    if not split_ag:
                        nc.gpsimd.collective_compute(
                            "AllGather",
                            mybir.AluOpType.bypass,
                            replica_groups=rg,
                            ins=[agin[:].opt()],
                            outs=[agout[:].opt()],
                        )
                        ofull = work.tile(
                            [128, NCH, SPAN], BF, tag="ofull", bufs=3, name=f"of{J}"
                        )
                        for k in range(NCH):
                            nc.sync.dma_start(
                                ofull[:, k, :], agout[128 * k : 128 * (k + 1), :]
                            )

                        def make_oproj(J=J, q0=q0, ofull=ofull):
                            def _emit():
                                for half in range(2):
                                    po = psS.tile(
                                        [128, SPAN], F32, tag="s", name=f"po{J}_{half}"
                                    )
                                    for k in range(NCH):
                                        nc.tensor.matmul(
                                            po[:],
                                            wo_sb[:, k, 128 * half : 128 * (half + 1)],
                                            ofull[:, k, :],
                                            start=(k == 0),
                                            stop=(k == NCH - 1),
                                        )
                                    outT = work.tile(
                                        [128, SPAN],
                                        BF,
                                        tag="outT",
                                        name=f"ot{J}_{half}",
                                    )
                                    nc.vector.tensor_copy(outT[:], po[:])
                                    nc.sync.dma_start(
                                        out[
                                            128 * half : 128 * (half + 1),
                                            q0 : q0 + SPAN,
                                        ],
                                        outT[:],
                                    )

                            return _emit

                        pending_oproj.append(make_oproj())
                        if len(pending_oproj) > 2:
                            pending_oproj.pop(0)()

                for fn in pending_oproj:
                    fn()

                # last span's output projection: accumulate per gathered
                # quarter so only the final quarter's AG latency is exposed
                q0 = SPAN * (NSPAN - 1)
                po3 = [
                    psS.tile([128, SPAN], F32, tag="s", name=f"po3_{half}")
                    for half in range(2)
                ]
                for q in range(4):  # q = 2*pr + hh, arrival order
                    pr, hh = q // 2, q % 2
                    for g in range(4):
                        h = 4 * g + 2 * pr + hh
                        k, r0 = h // 2, 64 * (h % 2)  # h % 2 == hh
                        rhs = of3[q][64 * hh : 64 * hh + 64, g, :]
                        for half in range(2):
                            nc.tensor.matmul(
                                po3[half][:],
                                wo_sb[r0 : r0 + 64, k, 128 * half : 128 * (half + 1)],
                                rhs,
                                start=(q == 0 and g == 0),
                                stop=(q == 3 and g == 3),
                            )
                for half in range(2):
                    outT = work.tile(
                        [128, SPAN], BF, tag="outT", name=f"ot3_{half}"
                    )
                    nc.vector.tensor_copy(outT[:], po3[half][:])
                    nc.sync.dma_start(
                        out[128 * half : 128 * (half + 1), q0 : q0 + SPAN],
                        outT[:],
                    )

    nc.finalize()
    return nc


def _chunked(w):
    # [128*k, c] -> [128, k, c]
    r, c = w.shape
    return np.ascontiguousarray(
        w.reshape(r // 128, 128, c).transpose(1, 0, 2)
    ).astype(BF16)


def _host_inputs(x, cos, sin, wq, wk, wv, wo):
    cosT = np.ascontiguousarray(cos.T).astype(np.float32)  # [64, S]
    sinT = np.ascontiguousarray(sin.T).astype(np.float32)
    c2n = np.concatenate([cosT, cosT], axis=0).astype(BF16)  # [128, S]
    s2n = np.ascontiguousarray(np.concatenate([sinT, sinT], axis=0)).astype(np.float32)
    cosT = cosT.astype(BF16)
    s1n = sinT.astype(np.float32)
    ident = np.eye(128, dtype=BF16)
    # upper-triangular (incl diagonal) keep-mask for the causal boundary block
    dmaskh = (np.arange(128)[None, :] >= np.arange(128)[:, None]).astype(BF16)
    # signed rotate-half permutation, block-diagonal per 64-dim head:
    # (prot^T @ q)[p] = -q[p+32] for p%64<32, +q[p-32] otherwise
    pr64 = np.zeros((64, 64), dtype=np.float32)
    pr64[32:64, 0:32] = -np.eye(32)
    pr64[0:32, 32:64] = np.eye(32)
    proth = np.zeros((128, 128), dtype=np.float32)
    proth[0:64, 0:64] = pr64
    proth[64:128, 64:128] = pr64
    proth = proth.astype(BF16)

    in_maps = []
    for c in range(N_CORES):
        b, g = c // 4, c % 4
        xTc = _chunked(np.ascontiguousarray(x[b].T))
        wq_c = _chunked(wq[:, 256 * g : 256 * (g + 1)] / 8.0)
        wkv_c = _chunked(
            np.concatenate(
                [wk[:, 64 * g : 64 * (g + 1)], wv[:, 64 * g : 64 * (g + 1)]], axis=1
            )
        )
        wo_c = _chunked(wo[:, 256 * g : 256 * (g + 1)])
        in_maps.append(
            {
                "xT": xTc,
                "wq": wq_c,
                "wkv": wkv_c,
                "wo": wo_c,
                "c2": c2n,
                "s2": s2n,
                "c1": cosT,
                "s1": s1n,
                "ident": ident,
                "dmask": dmaskh,
                "prot": proth,
            }
        )
    return in_maps


def kernel(x, cos, sin, wq, wk, wv, wo):
    if "nc" not in _CACHED:
        _CACHED["nc"] = _build_nc()
    nc = _CACHED["nc"]
    in_maps = _host_inputs(
        np.asarray(x, np.float32),
        np.asarray(cos, np.float32),
        np.asarray(sin, np.float32),
        np.asarray(wq, np.float32),
        np.asarray(wk, np.float32),
        np.asarray(wv, np.float32),
        np.asarray(wo, np.float32),
    )
    res = run_bass_kernel_spmd(
        nc, in_maps, core_ids=list(range(N_CORES)), trace=TRACE
    )
    _CACHED["last_result"] = res
    out = np.empty((B, S, HID), dtype=np.float32)
    for c in range(N_CORES):
        b, g = c // 4, c % 4
        out[b, :, 256 * g : 256 * (g + 1)] = res.results[c]["out"].T.astype(
            np.float32
        )
    return out
